# revision 3
# baseline (speedup 1.0000x reference)
"""Trainium2 Bass kernel for MAB (multihead attention block) — nn_MAB_48412871360901.

Data-parallel over batch: 16 batches -> 8 NeuronCores, 2 batches/core.

Design: minimize static instruction count via For_i hardware loops with
symbolic (register-offset) addressing. Host pre-transposes Q/K into
[dk%128, dk//128, n] layout so the device needs no transposes at all.
All matmul weights (lhsT) live at static SBUF offsets; data-dependent
lhsT operands (K^T chunks for the v-projection / S matmuls, v chunks for
the PV matmuls) are staged through fixed SBUF tiles with one DVE copy.

Per core (2 batches, n = 2048 rows):
  P2a  qT = Wq^T Q^T, kT = Wk^T K^T             (loop over 4 n-chunks)
  P2b  v  = K Wv, scattered into vaug with a ones column per head pair
       so the PV matmul also produces the softmax denominator Z
  P3   per (b, head-pair, q-chunk): loop over 8 k-chunks:
         S^T = k^T.T q^T (2 heads row-packed), exp (scale fused),
         PV matmuls (even head: [v|1] -> values + Z row; odd head:
         values at partitions 64:128, Z via ones-matmul), DVE f32
         accumulation across k-chunks; then 1/Z broadcast via matmul,
         X^T = U*(1/Z) + qT, SQ = X^2
  P4   LN0 in transposed layout (ones-matmul stats over partitions)
  P5   M^T = Wo^T Xn^T, relu, residual, LN1 transposed, bf16 out
Host casts the bf16 transposed output back to f32 natural layout.
"""

import sys
import numpy as np
import ml_dtypes

for _p in ("/opt/trn_rl_repo", "/root/.axon_site/_ro/trn_rl_repo"):
    if _p not in sys.path:
        sys.path.insert(0, _p)

import concourse.bacc as bacc
import concourse.mybir as mybir
import concourse.tile as tile
from concourse.bass_utils import run_bass_kernel_spmd

BF16 = mybir.dt.bfloat16
F32 = mybir.dt.float32
NBF = ml_dtypes.bfloat16
AF = mybir.ActivationFunctionType
OP = mybir.AluOpType

B, NQ, NK = 16, 1024, 1024
D = 512
H = 8
N_CORES = 8
BL = B // N_CORES          # batches per core
N = BL * NQ                # rows per core (2048)
EPS = 1e-5
SCALE = 1.0 / np.sqrt(512.0)

_cache = {}


def _build(flags, repeat=1):
    (bq_nz, bk_nz, bv_nz, bo_nz, ln0_aff, ln1_aff) = flags
    nc = bacc.Bacc("TRN2", target_bir_lowering=False, debug=False,
                   num_devices=N_CORES)

    dQT = nc.dram_tensor("QT", [128, 4, N], BF16, kind="ExternalInput").ap()
    dKT = nc.dram_tensor("KT", [128, 4, N], BF16, kind="ExternalInput").ap()
    dWQ = nc.dram_tensor("WQ", [128, 4, 4, 128], BF16, kind="ExternalInput").ap()
    dWK = nc.dram_tensor("WK", [128, 4, 4, 128], BF16, kind="ExternalInput").ap()
    dWV = nc.dram_tensor("WV", [128, 4, 512], BF16, kind="ExternalInput").ap()
    dWO = nc.dram_tensor("WO", [128, 4, 4, 128], BF16, kind="ExternalInput").ap()
    dOut = nc.dram_tensor("OUT", [128, 4, N], BF16, kind="ExternalOutput").ap()
    dBQ = nc.dram_tensor("BQ4", [128, 4], F32, kind="ExternalInput").ap() if bq_nz else None
    dBK = nc.dram_tensor("BK4", [128, 4], F32, kind="ExternalInput").ap() if bk_nz else None
    dBV = nc.dram_tensor("BVB", [128, 512], F32, kind="ExternalInput").ap() if bv_nz else None
    dBO = nc.dram_tensor("BO4", [128, 4], F32, kind="ExternalInput").ap() if bo_nz else None
    dG0 = nc.dram_tensor("G04", [128, 4], F32, kind="ExternalInput").ap() if ln0_aff else None
    dB0 = nc.dram_tensor("B04", [128, 4], F32, kind="ExternalInput").ap() if ln0_aff else None
    dG1 = nc.dram_tensor("G14", [128, 4], F32, kind="ExternalInput").ap() if ln1_aff else None
    dB1 = nc.dram_tensor("B14", [128, 4], F32, kind="ExternalInput").ap() if ln1_aff else None

    with tile.TileContext(nc) as tc:
        with tc.tile_pool(name="cst", bufs=1) as cst, \
             tc.tile_pool(name="stg", bufs=2) as stg, \
             tc.tile_pool(name="sml", bufs=1) as sml, \
             tc.tile_pool(name="tmp", bufs=2) as tmpp, \
             tc.tile_pool(name="psS", bufs=2, space="PSUM") as psS, \
             tc.tile_pool(name="psU", bufs=2, space="PSUM") as psU, \
             tc.tile_pool(name="psZ", bufs=2, space="PSUM") as psZ:

            # ---- constants / inputs (static DMAs) ----
            qt = cst.tile([128, 4, N], BF16, tag="qt")
            nc.sync.dma_start(out=qt, in_=dQT)
            kt = cst.tile([128, 4, N], BF16, tag="kt")
            nc.sync.dma_start(out=kt, in_=dKT)
            w_q = cst.tile([128, 4, 4, 128], BF16, tag="w_q")
            nc.sync.dma_start(out=w_q, in_=dWQ)
            w_k = cst.tile([128, 4, 4, 128], BF16, tag="w_k")
            nc.sync.dma_start(out=w_k, in_=dWK)
            w_v = cst.tile([128, 4, 512], BF16, tag="w_v")
            nc.sync.dma_start(out=w_v, in_=dWV)
            w_o = cst.tile([128, 4, 4, 128], BF16, tag="w_o")
            nc.sync.dma_start(out=w_o, in_=dWO)

            ones_b = cst.tile([128, 1], BF16, tag="ones_b")
            nc.vector.memset(ones_b, 1.0)
            ones_f = cst.tile([128, 64], F32, tag="ones_f")
            nc.vector.memset(ones_f, 1.0)
            ones_r = cst.tile([1, 128], F32, tag="ones_r")
            nc.vector.memset(ones_r, 1.0)
            eps1 = cst.tile([1, 1], F32, tag="eps1")
            nc.vector.memset(eps1, EPS)

            def ldf(dram, shape, tag):
                t = cst.tile(list(shape), F32, tag=tag)
                nc.sync.dma_start(out=t, in_=dram)
                return t

            bq4 = ldf(dBQ, (128, 4), "bq4") if bq_nz else None
            bk4 = ldf(dBK, (128, 4), "bk4") if bk_nz else None
            bvb = ldf(dBV, (128, 512), "bvb") if bv_nz else None
            bo4 = ldf(dBO, (128, 4), "bo4") if bo_nz else None
            g04 = ldf(dG0, (128, 4), "g04") if ln0_aff else None
            b04 = ldf(dB0, (128, 4), "b04") if ln0_aff else None
            g14 = ldf(dG1, (128, 4), "g14") if ln1_aff else None
            b14 = ldf(dB1, (128, 4), "b14") if ln1_aff else None

            # ---- persistent big tiles ----
            q_pT = cst.tile([128, 4, N], BF16, tag="q_pT")    # projected q^T
            k_pT = cst.tile([128, 4, N], BF16, tag="k_pT")    # projected k^T
            vaug = cst.tile([128, 16, 4, 130], BF16, tag="vaug")
            X = cst.tile([128, 4, N], BF16, tag="X")          # attn + q resid
            Xn = cst.tile([128, 4, N], BF16, tag="Xn")        # LN0 out
            xpre = cst.tile([128, 4, N], BF16, tag="xpre")    # Xn + relu(M)
            outT = cst.tile([128, 4, N], BF16, tag="outT")    # final out^T

            # views
            qt4 = qt.rearrange("p k (c q) -> p k c q", c=4)
            kt4 = kt.rearrange("p k (c q) -> p k c q", c=4)
            q5 = q_pT.rearrange("p d (b c q) -> p d b c q", b=2, c=2)
            qp4 = q_pT.rearrange("p d (c q) -> p d c q", c=4)
            kp4 = k_pT.rearrange("p d (c q) -> p d c q", c=4)
            k4 = kt.rearrange("p k (c q) -> p k c q", c=16)
            k5 = k_pT.rearrange("p d (b t q) -> p d b t q", b=2, t=8)
            vaug5 = vaug.rearrange("p (b t) h c -> p b t h c", b=2)
            X5 = X.rearrange("p d (b c q) -> p d b c q", b=2, c=2)
            X4 = X.rearrange("p d (c q) -> p d c q", c=4)
            Xn4 = Xn.rearrange("p d (c q) -> p d c q", c=4)
            xp4 = xpre.rearrange("p d (c q) -> p d c q", c=4)
            o4 = outT.rearrange("p d (c q) -> p d c q", c=4)

            # ones columns of vaug (col 64 of each 130-block) — memset whole
            nc.vector.memset(vaug, 1.0)

            def body():
                # ---- P2a: q/k projections (transposed out) ----
                with tc.For_i(0, 4, 1) as c:
                    for dstv, w, srcv, bias in ((qp4, w_q, qt4, bq4),
                                                (kp4, w_k, kt4, bk4)):
                        for dvt in range(4):
                            pp = psU.tile([128, 512], F32, tag="u")
                            for j in range(4):
                                nc.tensor.matmul(pp, lhsT=w[:, j, dvt, :],
                                                 rhs=srcv[:, j, c, :],
                                                 start=(j == 0), stop=(j == 3))
                            o = dstv[:, dvt, c, :]
                            if bias is not None:
                                nc.vector.tensor_scalar_add(
                                    out=o, in0=pp, scalar1=bias[:, dvt:dvt + 1])
                            else:
                                nc.vector.tensor_copy(out=o, in_=pp)

                # ---- P2b: v projection into vaug ----
                with tc.For_i(0, 16, 1) as ch:
                    kst = stg.tile([128, 4, 128], BF16, tag="kst")
                    for j in range(4):
                        nc.vector.tensor_copy(out=kst[:, j, :],
                                              in_=k4[:, j, ch, :])
                    pv = psU.tile([128, 512], F32, tag="u")
                    for j in range(4):
                        nc.tensor.matmul(pv, lhsT=kst[:, j, :],
                                         rhs=w_v[:, j, :],
                                         start=(j == 0), stop=(j == 3))
                    pvv = pv.rearrange("p (h e c) -> p h e c", h=4, e=2)
                    if bvb is not None:
                        bvv = bvb.rearrange("p (h e c) -> p h e c", h=4, e=2)
                        nc.vector.tensor_tensor(
                            out=vaug[:, ch, :, 0:64], in0=pvv[:, :, 0, :],
                            in1=bvv[:, :, 0, :], op=OP.add)
                        nc.vector.tensor_tensor(
                            out=vaug[:, ch, :, 65:129], in0=pvv[:, :, 1, :],
                            in1=bvv[:, :, 1, :], op=OP.add)
                    else:
                        nc.vector.tensor_copy(out=vaug[:, ch, :, 0:64],
                                              in_=pvv[:, :, 0, :])
                        nc.vector.tensor_copy(out=vaug[:, ch, :, 65:129],
                                              in_=pvv[:, :, 1, :])

                # ---- P3: attention ----
                with tc.For_i(0, 2, 1) as b:
                    with tc.For_i(0, 4, 1) as hp:
                        with tc.For_i(0, 2, 1) as qc:
                            ps_uE = psU.tile([128, 512], F32, tag="u")
                            ps_uO = psU.tile([128, 512], F32, tag="u")
                            ps_zO = psZ.tile([1, 512], F32, tag="z")
                            for kt_i in range(8):
                                kstg = stg.tile([128, 128], BF16, tag="kstg")
                                nc.vector.tensor_copy(out=kstg,
                                                      in_=k5[:, hp, b, kt_i, :])
                                ps_s = psS.tile([128, 1024], F32, tag="s")
                                nc.tensor.matmul(
                                    ps_s[:, 0:512], lhsT=kstg[0:64, :],
                                    rhs=q5[0:64, hp, b, qc, :],
                                    start=True, stop=True, tile_position=(0, 0))
                                nc.tensor.matmul(
                                    ps_s[:, 512:1024], lhsT=kstg[64:128, :],
                                    rhs=q5[64:128, hp, b, qc, :],
                                    start=True, stop=True, tile_position=(64, 0))
                                ex = stg.tile([128, 1024], BF16, tag="ex")
                                nc.scalar.activation(out=ex, in_=ps_s,
                                                     func=AF.Exp, scale=SCALE)
                                vst = stg.tile([128, 130], BF16, tag="vst")
                                nc.vector.tensor_copy(out=vst,
                                                      in_=vaug5[:, b, kt_i, hp, :])
                                nc.tensor.matmul(ps_uE[0:65, :],
                                                 lhsT=vst[:, 0:65],
                                                 rhs=ex[:, 0:512],
                                                 start=(kt_i == 0),
                                                 stop=(kt_i == 7))
                                nc.tensor.matmul(ps_uO[64:128, :],
                                                 lhsT=vst[:, 65:129],
                                                 rhs=ex[:, 512:1024],
                                                 start=(kt_i == 0),
                                                 stop=(kt_i == 7))
                                nc.tensor.matmul(ps_zO, lhsT=ones_b,
                                                 rhs=ex[:, 512:1024],
                                                 start=(kt_i == 0),
                                                 stop=(kt_i == 7))
                            # softmax normalize + residual
                            zr = sml.tile([128, 512], F32, tag="zr")
                            nc.vector.reciprocal(out=zr[64:65, :],
                                                 in_=ps_uE[64:65, :])
                            nc.vector.reciprocal(out=zr[0:1, :], in_=ps_zO)
                            bc = psS.tile([128, 1024], F32, tag="s")
                            nc.tensor.matmul(bc[0:64, 0:512],
                                             lhsT=ones_f[64:65, 0:64],
                                             rhs=zr[64:65, :],
                                             start=True, stop=True)
                            nc.tensor.matmul(bc[64:128, 0:512],
                                             lhsT=ones_f[0:1, 0:64],
                                             rhs=zr[0:1, :],
                                             start=True, stop=True)
                            bcs = tmpp.tile([128, 512], BF16, tag="bcs")
                            nc.vector.tensor_copy(out=bcs, in_=bc[:, 0:512])
                            tt = tmpp.tile([128, 512], BF16, tag="tt")
                            nc.vector.tensor_tensor(out=tt[0:64, :],
                                                    in0=ps_uE[0:64, :],
                                                    in1=bcs[0:64, :],
                                                    op=OP.mult)
                            nc.vector.tensor_tensor(out=tt[64:128, :],
                                                    in0=ps_uO[64:128, :],
                                                    in1=bcs[64:128, :],
                                                    op=OP.mult)
                            xs = X5[:, hp, b, qc, :]
                            nc.vector.tensor_tensor(out=xs, in0=tt,
                                                    in1=q5[:, hp, b, qc, :],
                                                    op=OP.add)

                # ---- P4: LN0 (transposed layout) ----
                with tc.For_i(0, 4, 1) as c:
                    st_x = psZ.tile([1, 512], F32, tag="z")
                    st_s = psZ.tile([1, 512], F32, tag="z")
                    for dvt in range(4):
                        nc.tensor.matmul(st_x, lhsT=ones_b,
                                         rhs=X4[:, dvt, c, :],
                                         start=(dvt == 0), stop=(dvt == 3))
                        sq0 = tmpp.tile([128, 512], BF16, tag="sq0")
                        nc.vector.tensor_tensor(out=sq0, in0=X4[:, dvt, c, :],
                                                in1=X4[:, dvt, c, :],
                                                op=OP.mult)
                        nc.tensor.matmul(st_s, lhsT=ones_b, rhs=sq0,
                                         start=(dvt == 0), stop=(dvt == 3))
                    mu = sml.tile([1, 512], F32, tag="mu")
                    nc.vector.tensor_scalar_mul(out=mu, in0=st_x,
                                                scalar1=1.0 / D)
                    mu2 = sml.tile([1, 512], F32, tag="mu2")
                    nc.vector.tensor_tensor(out=mu2, in0=mu, in1=mu,
                                            op=OP.mult)
                    var = sml.tile([1, 512], F32, tag="var")
                    nc.vector.scalar_tensor_tensor(out=var, in0=st_s,
                                                   scalar=1.0 / D, in1=mu2,
                                                   op0=OP.mult,
                                                   op1=OP.subtract)
                    lnv = sml.tile([1, 512], F32, tag="lnv")
                    nc.scalar.activation(out=lnv, in_=var, func=AF.Ln,
                                         bias=eps1, scale=1.0)
                    rstd = sml.tile([1, 512], F32, tag="rstd")
                    nc.scalar.activation(out=rstd, in_=lnv, func=AF.Exp,
                                         scale=-0.5)
                    nmr = sml.tile([1, 512], F32, tag="nmr")
                    nc.vector.scalar_tensor_tensor(out=nmr, in0=mu,
                                                   scalar=-1.0, in1=rstd,
                                                   op0=OP.mult, op1=OP.mult)
                    ps_b = psS.tile([128, 1024], F32, tag="s")
                    nc.tensor.matmul(ps_b[:, 0:512], lhsT=ones_r, rhs=rstd, start=True, stop=True)
                    nc.tensor.matmul(ps_b[:, 512:1024], lhsT=ones_r, rhs=nmr, start=True, stop=True)
                    for dvt in range(4):
                        t2 = tmpp.tile([128, 512], BF16, tag="t2")
                        nc.vector.tensor_tensor(out=t2, in0=X4[:, dvt, c, :],
                                                in1=ps_b[:, 0:512], op=OP.mult)
                        xn = Xn4[:, dvt, c, :]
                        nc.vector.tensor_tensor(out=xn, in0=t2,
                                                in1=ps_b[:, 512:1024],
                                                op=OP.add)
                        if ln0_aff:
                            nc.vector.tensor_scalar(
                                out=xn, in0=xn,
                                scalar1=g04[:, dvt:dvt + 1],
                                scalar2=b04[:, dvt:dvt + 1],
                                op0=OP.mult, op1=OP.add)

                # ---- P5: Wo, relu, residual, LN1 (transposed), out ----
                with tc.For_i(0, 4, 1) as c:
                    st_x = psZ.tile([1, 512], F32, tag="z")
                    st_s = psZ.tile([1, 512], F32, tag="z")
                    for dv2t in range(4):
                        ps_m = psU.tile([128, 512], F32, tag="u")
                        for j in range(4):
                            nc.tensor.matmul(ps_m, lhsT=w_o[:, j, dv2t, :],
                                             rhs=Xn4[:, j, c, :],
                                             start=(j == 0), stop=(j == 3))
                        rl = tmpp.tile([128, 512], BF16, tag="rl")
                        if bo4 is not None:
                            nc.vector.tensor_scalar(
                                out=rl, in0=ps_m,
                                scalar1=bo4[:, dv2t:dv2t + 1], scalar2=0.0,
                                op0=OP.add, op1=OP.max)
                        else:
                            nc.vector.tensor_scalar_max(out=rl, in0=ps_m,
                                                        scalar1=0.0)
                        xp = xp4[:, dv2t, c, :]
                        nc.vector.tensor_tensor(out=xp, in0=rl,
                                                in1=Xn4[:, dv2t, c, :],
                                                op=OP.add)
                        sq = tmpp.tile([128, 512], BF16, tag="sq")
                        nc.vector.tensor_tensor(out=sq, in0=xp, in1=xp,
                                                op=OP.mult)
                        nc.tensor.matmul(st_x, lhsT=ones_b, rhs=xp,
                                         start=(dv2t == 0), stop=(dv2t == 3))
                        nc.tensor.matmul(st_s, lhsT=ones_b, rhs=sq,
                                         start=(dv2t == 0), stop=(dv2t == 3))
                    mu = sml.tile([1, 512], F32, tag="mu")
                    nc.vector.tensor_scalar_mul(out=mu, in0=st_x,
                                                scalar1=1.0 / D)
                    mu2 = sml.tile([1, 512], F32, tag="mu2")
                    nc.vector.tensor_tensor(out=mu2, in0=mu, in1=mu,
                                            op=OP.mult)
                    var = sml.tile([1, 512], F32, tag="var")
                    nc.vector.scalar_tensor_tensor(out=var, in0=st_s,
                                                   scalar=1.0 / D, in1=mu2,
                                                   op0=OP.mult,
                                                   op1=OP.subtract)
                    lnv = sml.tile([1, 512], F32, tag="lnv")
                    nc.scalar.activation(out=lnv, in_=var, func=AF.Ln,
                                         bias=eps1, scale=1.0)
                    rstd = sml.tile([1, 512], F32, tag="rstd")
                    nc.scalar.activation(out=rstd, in_=lnv, func=AF.Exp,
                                         scale=-0.5)
                    nmr = sml.tile([1, 512], F32, tag="nmr")
                    nc.vector.scalar_tensor_tensor(out=nmr, in0=mu,
                                                   scalar=-1.0, in1=rstd,
                                                   op0=OP.mult, op1=OP.mult)
                    ps_b = psS.tile([128, 1024], F32, tag="s")
                    nc.tensor.matmul(ps_b[:, 0:512], lhsT=ones_r, rhs=rstd, start=True, stop=True)
                    nc.tensor.matmul(ps_b[:, 512:1024], lhsT=ones_r, rhs=nmr, start=True, stop=True)
                    for dv2t in range(4):
                        t2 = tmpp.tile([128, 512], BF16, tag="t2")
                        nc.vector.tensor_tensor(out=t2, in0=xp4[:, dv2t, c, :],
                                                in1=ps_b[:, 0:512], op=OP.mult)
                        oo = o4[:, dv2t, c, :]
                        nc.vector.tensor_tensor(out=oo, in0=t2,
                                                in1=ps_b[:, 512:1024],
                                                op=OP.add)
                        if ln1_aff:
                            nc.vector.tensor_scalar(
                                out=oo, in0=oo,
                                scalar1=g14[:, dv2t:dv2t + 1],
                                scalar2=b14[:, dv2t:dv2t + 1],
                                op0=OP.mult, op1=OP.add)

                nc.sync.dma_start(out=dOut, in_=outT)

            if repeat == 1:
                body()
            else:
                with tc.For_i(0, repeat, 1):
                    body()

    nc.compile()
    return nc


def _host_T(x):
    # [n, d] f32 -> [128, 4, n] bf16 with d = dt*128 + p
    n = x.shape[0]
    return np.ascontiguousarray(
        x.T.reshape(4, 128, n).transpose(1, 0, 2).astype(NBF))


def _consts(Wq, Wk, Wv, Wo, flags, bq, bk, bv, bo, g0, b0, g1, b1):
    (bq_nz, bk_nz, bv_nz, bo_nz, ln0_aff, ln1_aff) = flags

    def wblk(W):
        # [512, 512] -> [128, 4kt, 4dvt, 128] bf16,
        # lhsT block [p, kt, dvt, c] = W[kt*128+p, dvt*128+c]
        return np.ascontiguousarray(
            np.asarray(W).reshape(4, 128, 4, 128).transpose(1, 0, 2, 3)
            .astype(NBF))

    c = {
        "WQ": wblk(Wq),
        "WK": wblk(Wk),
        "WO": wblk(Wo),
        "WV": np.ascontiguousarray(
            np.asarray(Wv).reshape(4, 128, 512).transpose(1, 0, 2).astype(NBF)),
    }
    def p4(v):
        return np.ascontiguousarray(
            np.asarray(v, np.float32).reshape(4, 128).T)
    if bq_nz: c["BQ4"] = p4(bq)
    if bk_nz: c["BK4"] = p4(bk)
    if bv_nz: c["BVB"] = np.ascontiguousarray(
        np.broadcast_to(np.asarray(bv, np.float32), (128, 512)))
    if bo_nz: c["BO4"] = p4(bo)
    if ln0_aff:
        c["G04"] = p4(g0)
        c["B04"] = p4(b0)
    if ln1_aff:
        c["G14"] = p4(g1)
        c["B14"] = p4(b1)
    return c


def make_in_maps(Q, K, Wq, bq, Wk, bk, Wv, bv, Wo, bo, g0, b0, g1, b1, flags):
    consts = _consts(Wq, Wk, Wv, Wo, flags, bq, bk, bv, bo, g0, b0, g1, b1)
    Qf = np.asarray(Q, np.float32).reshape(B, NQ, 512)
    Kf = np.asarray(K, np.float32).reshape(B, NK, 512)
    in_maps = []
    for ci in range(N_CORES):
        m = dict(consts)
        m["QT"] = _host_T(Qf[ci * BL:(ci + 1) * BL].reshape(N, 512))
        m["KT"] = _host_T(Kf[ci * BL:(ci + 1) * BL].reshape(N, 512))
        in_maps.append(m)
    return in_maps


def get_flags(bq, bk, bv, bo, g0, b0, g1, b1):
    return (bool(np.any(np.asarray(bq))), bool(np.any(np.asarray(bk))),
            bool(np.any(np.asarray(bv))), bool(np.any(np.asarray(bo))),
            bool(np.any(np.asarray(g0) != 1) or np.any(np.asarray(b0))),
            bool(np.any(np.asarray(g1) != 1) or np.any(np.asarray(b1))))


def get_program(flags, repeat=1):
    key = (flags, repeat)
    if key not in _cache:
        _cache[key] = _build(flags, repeat)
    return _cache[key]


def kernel(Q, K, Wq, bq, Wk, bk, Wv, bv, Wo, bo, g0, b0, g1, b1):
    flags = get_flags(bq, bk, bv, bo, g0, b0, g1, b1)
    nc = get_program(flags, repeat=1)
    in_maps = make_in_maps(Q, K, Wq, bq, Wk, bk, Wv, bv, Wo, bo,
                           g0, b0, g1, b1, flags)
    res = run_bass_kernel_spmd(nc, in_maps, list(range(N_CORES)))
    out = np.empty((B, NQ, D), np.float32)
    for ci in range(N_CORES):
        o = np.asarray(res.results[ci]["OUT"]).astype(np.float32)
        # [128, 4, N] -> [N, 512] with d = dt*128 + p
        out[ci * BL:(ci + 1) * BL] = (
            o.transpose(1, 0, 2).reshape(512, N).T.reshape(BL, NQ, D))
    return out


# revision 4
# speedup vs baseline: 1.0413x; 1.0413x over previous
"""Trainium2 Bass kernel for MAB (multihead attention block) — nn_MAB_48412871360901.

Data-parallel over batch: 16 batches -> 8 NeuronCores, 2 batches/core.

Design: minimize static instruction count via For_i hardware loops with
symbolic (register-offset) addressing. Host pre-transposes Q/K into
[dk%128, dk//128, n] layout so the device needs no transposes at all.
All matmul weights (lhsT) live at static SBUF offsets; data-dependent
lhsT operands (K^T chunks for the v-projection / S matmuls, v chunks for
the PV matmuls) are staged through fixed SBUF tiles with one DVE copy.

Per core (2 batches, n = 2048 rows):
  P2a  qT = Wq^T Q^T, kT = Wk^T K^T             (loop over 4 n-chunks)
  P2b  v  = K Wv, scattered into vaug with a ones column per head pair
       so the PV matmul also produces the softmax denominator Z
  P3   per (b, head-pair, q-chunk): loop over 8 k-chunks:
         S^T = k^T.T q^T (2 heads row-packed), exp (scale fused),
         PV matmuls (even head: [v|1] -> values + Z row; odd head:
         values at partitions 64:128, Z via ones-matmul), DVE f32
         accumulation across k-chunks; then 1/Z broadcast via matmul,
         X^T = U*(1/Z) + qT, SQ = X^2
  P4   LN0 in transposed layout (ones-matmul stats over partitions)
  P5   M^T = Wo^T Xn^T, relu, residual, LN1 transposed, bf16 out
Host casts the bf16 transposed output back to f32 natural layout.
"""

import sys
import numpy as np
import ml_dtypes

for _p in ("/opt/trn_rl_repo", "/root/.axon_site/_ro/trn_rl_repo"):
    if _p not in sys.path:
        sys.path.insert(0, _p)

import concourse.bacc as bacc
import concourse.mybir as mybir
import concourse.tile as tile
from concourse.bass_utils import run_bass_kernel_spmd

BF16 = mybir.dt.bfloat16
F32 = mybir.dt.float32
NBF = ml_dtypes.bfloat16
AF = mybir.ActivationFunctionType
OP = mybir.AluOpType

B, NQ, NK = 16, 1024, 1024
D = 512
H = 8
N_CORES = 8
BL = B // N_CORES          # batches per core
N = BL * NQ                # rows per core (2048)
EPS = 1e-5
SCALE = 1.0 / np.sqrt(512.0)

_cache = {}


def _build(flags, repeat=1):
    (bq_nz, bk_nz, bv_nz, bo_nz, ln0_aff, ln1_aff) = flags
    nc = bacc.Bacc("TRN2", target_bir_lowering=False, debug=False,
                   num_devices=N_CORES)

    dQT = nc.dram_tensor("QT", [128, 4, N], BF16, kind="ExternalInput").ap()
    dKT = nc.dram_tensor("KT", [128, 4, N], BF16, kind="ExternalInput").ap()
    dWQ = nc.dram_tensor("WQ", [128, 4, 4, 128], BF16, kind="ExternalInput").ap()
    dWK = nc.dram_tensor("WK", [128, 4, 4, 128], BF16, kind="ExternalInput").ap()
    dWV = nc.dram_tensor("WV", [128, 4, 512], BF16, kind="ExternalInput").ap()
    dWO = nc.dram_tensor("WO", [128, 4, 4, 128], BF16, kind="ExternalInput").ap()
    dOut = nc.dram_tensor("OUT", [128, 4, N], BF16, kind="ExternalOutput").ap()
    dBQ = nc.dram_tensor("BQ4", [128, 4], F32, kind="ExternalInput").ap() if bq_nz else None
    dBK = nc.dram_tensor("BK4", [128, 4], F32, kind="ExternalInput").ap() if bk_nz else None
    dBV = nc.dram_tensor("BVB", [128, 512], F32, kind="ExternalInput").ap() if bv_nz else None
    dBO = nc.dram_tensor("BO4", [128, 4], F32, kind="ExternalInput").ap() if bo_nz else None
    dG0 = nc.dram_tensor("G04", [128, 4], F32, kind="ExternalInput").ap() if ln0_aff else None
    dB0 = nc.dram_tensor("B04", [128, 4], F32, kind="ExternalInput").ap() if ln0_aff else None
    dG1 = nc.dram_tensor("G14", [128, 4], F32, kind="ExternalInput").ap() if ln1_aff else None
    dB1 = nc.dram_tensor("B14", [128, 4], F32, kind="ExternalInput").ap() if ln1_aff else None

    with tile.TileContext(nc) as tc:
        with tc.tile_pool(name="cst", bufs=1) as cst, \
             tc.tile_pool(name="stg", bufs=2) as stg, \
             tc.tile_pool(name="sml", bufs=1) as sml, \
             tc.tile_pool(name="tmp", bufs=2) as tmpp, \
             tc.tile_pool(name="psS", bufs=2, space="PSUM") as psS, \
             tc.tile_pool(name="psU", bufs=2, space="PSUM") as psU, \
             tc.tile_pool(name="psZ", bufs=2, space="PSUM") as psZ:

            # ---- constants / inputs (static DMAs) ----
            qt = cst.tile([128, 4, N], BF16, tag="qt")
            nc.sync.dma_start(out=qt, in_=dQT)
            kt = cst.tile([128, 4, N], BF16, tag="kt")
            nc.sync.dma_start(out=kt, in_=dKT)
            w_q = cst.tile([128, 4, 4, 128], BF16, tag="w_q")
            nc.sync.dma_start(out=w_q, in_=dWQ)
            w_k = cst.tile([128, 4, 4, 128], BF16, tag="w_k")
            nc.sync.dma_start(out=w_k, in_=dWK)
            w_v = cst.tile([128, 4, 512], BF16, tag="w_v")
            nc.sync.dma_start(out=w_v, in_=dWV)
            w_o = cst.tile([128, 4, 4, 128], BF16, tag="w_o")
            nc.sync.dma_start(out=w_o, in_=dWO)

            ones_b = cst.tile([128, 1], BF16, tag="ones_b")
            nc.vector.memset(ones_b, 1.0)
            ones_f = cst.tile([128, 64], F32, tag="ones_f")
            nc.vector.memset(ones_f, 1.0)
            ones_r = cst.tile([1, 128], F32, tag="ones_r")
            nc.vector.memset(ones_r, 1.0)
            eps1 = cst.tile([1, 1], F32, tag="eps1")
            nc.vector.memset(eps1, EPS)

            def ldf(dram, shape, tag):
                t = cst.tile(list(shape), F32, tag=tag)
                nc.sync.dma_start(out=t, in_=dram)
                return t

            bq4 = ldf(dBQ, (128, 4), "bq4") if bq_nz else None
            bk4 = ldf(dBK, (128, 4), "bk4") if bk_nz else None
            bvb = ldf(dBV, (128, 512), "bvb") if bv_nz else None
            bo4 = ldf(dBO, (128, 4), "bo4") if bo_nz else None
            g04 = ldf(dG0, (128, 4), "g04") if ln0_aff else None
            b04 = ldf(dB0, (128, 4), "b04") if ln0_aff else None
            g14 = ldf(dG1, (128, 4), "g14") if ln1_aff else None
            b14 = ldf(dB1, (128, 4), "b14") if ln1_aff else None

            # ---- persistent big tiles ----
            q_pT = cst.tile([128, 4, N], BF16, tag="q_pT")    # projected q^T
            k_pT = cst.tile([128, 4, N], BF16, tag="k_pT")    # projected k^T
            vaug = cst.tile([128, 16, 4, 130], BF16, tag="vaug")
            X = cst.tile([128, 4, N], BF16, tag="X")          # attn + q resid
            Xn = cst.tile([128, 4, N], BF16, tag="Xn")        # LN0 out
            xpre = cst.tile([128, 4, N], BF16, tag="xpre")    # Xn + relu(M)
            outT = cst.tile([128, 4, N], BF16, tag="outT")    # final out^T

            # views
            qt4 = qt.rearrange("p k (c q) -> p k c q", c=4)
            kt4 = kt.rearrange("p k (c q) -> p k c q", c=4)
            q5 = q_pT.rearrange("p d (b c q) -> p d b c q", b=2, c=2)
            qp4 = q_pT.rearrange("p d (c q) -> p d c q", c=4)
            kp4 = k_pT.rearrange("p d (c q) -> p d c q", c=4)
            k4u = kt.rearrange("p k (c u q) -> p k c u q", c=8, u=2)
            vaug_u = vaug.rearrange("p (c u) h w -> p c u h w", c=8)
            k5 = k_pT.rearrange("p d (b t q) -> p d b t q", b=2, t=8)
            vaug5 = vaug.rearrange("p (b t) h c -> p b t h c", b=2)
            X5 = X.rearrange("p d (b c q) -> p d b c q", b=2, c=2)
            X4 = X.rearrange("p d (c q) -> p d c q", c=4)
            Xn4 = Xn.rearrange("p d (c q) -> p d c q", c=4)
            xp4 = xpre.rearrange("p d (c q) -> p d c q", c=4)
            o4 = outT.rearrange("p d (c q) -> p d c q", c=4)

            # ones columns of vaug (col 64 of each 130-block) — memset whole
            nc.vector.memset(vaug, 1.0)

            def body():
                # ---- P2a: q/k projections (transposed out) ----
                with tc.For_i(0, 4, 1) as c:
                    for dstv, w, srcv, bias in ((qp4, w_q, qt4, bq4),
                                                (kp4, w_k, kt4, bk4)):
                        for dvt in range(4):
                            pp = psU.tile([128, 512], F32, tag="u")
                            for j in range(4):
                                nc.tensor.matmul(pp, lhsT=w[:, j, dvt, :],
                                                 rhs=srcv[:, j, c, :],
                                                 start=(j == 0), stop=(j == 3))
                            o = dstv[:, dvt, c, :]
                            if bias is not None:
                                nc.vector.tensor_scalar_add(
                                    out=o, in0=pp, scalar1=bias[:, dvt:dvt + 1])
                            else:
                                nc.vector.tensor_copy(out=o, in_=pp)

                # ---- P2b: v projection into vaug ----
                with tc.For_i(0, 8, 1) as ch2:
                    for u in range(2):
                        kst = stg.tile([128, 4, 128], BF16, tag="kst")
                        for j in range(4):
                            nc.vector.tensor_copy(out=kst[:, j, :],
                                                  in_=k4u[:, j, ch2, u, :])
                        pv = psU.tile([128, 512], F32, tag="u")
                        for j in range(4):
                            nc.tensor.matmul(pv, lhsT=kst[:, j, :],
                                             rhs=w_v[:, j, :],
                                             start=(j == 0), stop=(j == 3))
                        pvv = pv.rearrange("p (h e c) -> p h e c", h=4, e=2)
                        if bvb is not None:
                            bvv = bvb.rearrange("p (h e c) -> p h e c", h=4, e=2)
                            nc.vector.tensor_tensor(
                                out=vaug_u[:, ch2, u, :, 0:64],
                                in0=pvv[:, :, 0, :],
                                in1=bvv[:, :, 0, :], op=OP.add)
                            nc.vector.tensor_tensor(
                                out=vaug_u[:, ch2, u, :, 65:129],
                                in0=pvv[:, :, 1, :],
                                in1=bvv[:, :, 1, :], op=OP.add)
                        else:
                            nc.vector.tensor_copy(out=vaug_u[:, ch2, u, :, 0:64],
                                                  in_=pvv[:, :, 0, :])
                            nc.vector.tensor_copy(out=vaug_u[:, ch2, u, :, 65:129],
                                                  in_=pvv[:, :, 1, :])

                # ---- P3: attention ----
                with tc.For_i(0, 2, 1) as b:
                    with tc.For_i(0, 4, 1) as hp:
                        kstg8 = stg.tile([128, 8, 128], BF16, tag="kstg8")
                        vst8 = stg.tile([128, 8, 130], BF16, tag="vst8")
                        for kt_i in range(8):
                            nc.vector.tensor_copy(out=kstg8[:, kt_i, :],
                                                  in_=k5[:, hp, b, kt_i, :])
                            nc.vector.tensor_copy(out=vst8[:, kt_i, :],
                                                  in_=vaug5[:, b, kt_i, hp, :])
                        for qc_i in range(2):
                            ps_uE = psU.tile([128, 512], F32, tag="u")
                            ps_uO = psU.tile([128, 512], F32, tag="u")
                            ps_zO = psZ.tile([1, 512], F32, tag="z")
                            for kt_i in range(8):
                                ps_s = psS.tile([128, 1024], F32, tag="s")
                                nc.tensor.matmul(
                                    ps_s[:, 0:512], lhsT=kstg8[0:64, kt_i, :],
                                    rhs=q5[0:64, hp, b, qc_i, :],
                                    start=True, stop=True, tile_position=(0, 0))
                                nc.tensor.matmul(
                                    ps_s[:, 512:1024],
                                    lhsT=kstg8[64:128, kt_i, :],
                                    rhs=q5[64:128, hp, b, qc_i, :],
                                    start=True, stop=True, tile_position=(64, 0))
                                ex = stg.tile([128, 1024], BF16, tag="ex")
                                nc.scalar.activation(out=ex, in_=ps_s,
                                                     func=AF.Exp, scale=SCALE)
                                nc.tensor.matmul(ps_uE[0:65, :],
                                                 lhsT=vst8[:, kt_i, 0:65],
                                                 rhs=ex[:, 0:512],
                                                 start=(kt_i == 0),
                                                 stop=(kt_i == 7))
                                nc.tensor.matmul(ps_uO[64:128, :],
                                                 lhsT=vst8[:, kt_i, 65:129],
                                                 rhs=ex[:, 512:1024],
                                                 start=(kt_i == 0),
                                                 stop=(kt_i == 7))
                                nc.tensor.matmul(ps_zO, lhsT=ones_b,
                                                 rhs=ex[:, 512:1024],
                                                 start=(kt_i == 0),
                                                 stop=(kt_i == 7))
                            # softmax normalize + residual
                            zr = sml.tile([128, 512], F32, tag="zr")
                            nc.vector.reciprocal(out=zr[64:65, :],
                                                 in_=ps_uE[64:65, :])
                            nc.vector.reciprocal(out=zr[0:1, :], in_=ps_zO)
                            bc = psS.tile([128, 1024], F32, tag="s")
                            nc.tensor.matmul(bc[0:64, 0:512],
                                             lhsT=ones_f[64:65, 0:64],
                                             rhs=zr[64:65, :],
                                             start=True, stop=True)
                            nc.tensor.matmul(bc[64:128, 0:512],
                                             lhsT=ones_f[0:1, 0:64],
                                             rhs=zr[0:1, :],
                                             start=True, stop=True)
                            bcs = tmpp.tile([128, 512], BF16, tag="bcs")
                            nc.vector.tensor_copy(out=bcs, in_=bc[:, 0:512])
                            tt = tmpp.tile([128, 512], BF16, tag="tt")
                            nc.vector.tensor_tensor(out=tt[0:64, :],
                                                    in0=ps_uE[0:64, :],
                                                    in1=bcs[0:64, :],
                                                    op=OP.mult)
                            nc.vector.tensor_tensor(out=tt[64:128, :],
                                                    in0=ps_uO[64:128, :],
                                                    in1=bcs[64:128, :],
                                                    op=OP.mult)
                            xs = X5[:, hp, b, qc_i, :]
                            nc.vector.tensor_tensor(out=xs, in0=tt,
                                                    in1=q5[:, hp, b, qc_i, :],
                                                    op=OP.add)

                # ---- P4: LN0 (transposed layout) ----
                with tc.For_i(0, 4, 1) as c:
                    st_x = psZ.tile([1, 512], F32, tag="z")
                    st_s = psZ.tile([1, 512], F32, tag="z")
                    for dvt in range(4):
                        nc.tensor.matmul(st_x, lhsT=ones_b,
                                         rhs=X4[:, dvt, c, :],
                                         start=(dvt == 0), stop=(dvt == 3))
                        sq0 = tmpp.tile([128, 512], BF16, tag="sq0")
                        nc.vector.tensor_tensor(out=sq0, in0=X4[:, dvt, c, :],
                                                in1=X4[:, dvt, c, :],
                                                op=OP.mult)
                        nc.tensor.matmul(st_s, lhsT=ones_b, rhs=sq0,
                                         start=(dvt == 0), stop=(dvt == 3))
                    mu = sml.tile([1, 512], F32, tag="mu")
                    nc.vector.tensor_scalar_mul(out=mu, in0=st_x,
                                                scalar1=1.0 / D)
                    mu2 = sml.tile([1, 512], F32, tag="mu2")
                    nc.vector.tensor_tensor(out=mu2, in0=mu, in1=mu,
                                            op=OP.mult)
                    var = sml.tile([1, 512], F32, tag="var")
                    nc.vector.scalar_tensor_tensor(out=var, in0=st_s,
                                                   scalar=1.0 / D, in1=mu2,
                                                   op0=OP.mult,
                                                   op1=OP.subtract)
                    lnv = sml.tile([1, 512], F32, tag="lnv")
                    nc.scalar.activation(out=lnv, in_=var, func=AF.Ln,
                                         bias=eps1, scale=1.0)
                    rstd = sml.tile([1, 512], F32, tag="rstd")
                    nc.scalar.activation(out=rstd, in_=lnv, func=AF.Exp,
                                         scale=-0.5)
                    nmr = sml.tile([1, 512], F32, tag="nmr")
                    nc.vector.scalar_tensor_tensor(out=nmr, in0=mu,
                                                   scalar=-1.0, in1=rstd,
                                                   op0=OP.mult, op1=OP.mult)
                    ps_b = psS.tile([128, 1024], F32, tag="s")
                    nc.tensor.matmul(ps_b[:, 0:512], lhsT=ones_r, rhs=rstd, start=True, stop=True)
                    nc.tensor.matmul(ps_b[:, 512:1024], lhsT=ones_r, rhs=nmr, start=True, stop=True)
                    for dvt in range(4):
                        t2 = tmpp.tile([128, 512], BF16, tag="t2")
                        nc.vector.tensor_tensor(out=t2, in0=X4[:, dvt, c, :],
                                                in1=ps_b[:, 0:512], op=OP.mult)
                        xn = Xn4[:, dvt, c, :]
                        nc.vector.tensor_tensor(out=xn, in0=t2,
                                                in1=ps_b[:, 512:1024],
                                                op=OP.add)
                        if ln0_aff:
                            nc.vector.tensor_scalar(
                                out=xn, in0=xn,
                                scalar1=g04[:, dvt:dvt + 1],
                                scalar2=b04[:, dvt:dvt + 1],
                                op0=OP.mult, op1=OP.add)

                # ---- P5: Wo, relu, residual, LN1 (transposed), out ----
                with tc.For_i(0, 4, 1) as c:
                    st_x = psZ.tile([1, 512], F32, tag="z")
                    st_s = psZ.tile([1, 512], F32, tag="z")
                    for dv2t in range(4):
                        ps_m = psU.tile([128, 512], F32, tag="u")
                        for j in range(4):
                            nc.tensor.matmul(ps_m, lhsT=w_o[:, j, dv2t, :],
                                             rhs=Xn4[:, j, c, :],
                                             start=(j == 0), stop=(j == 3))
                        rl = tmpp.tile([128, 512], BF16, tag="rl")
                        if bo4 is not None:
                            nc.vector.tensor_scalar(
                                out=rl, in0=ps_m,
                                scalar1=bo4[:, dv2t:dv2t + 1], scalar2=0.0,
                                op0=OP.add, op1=OP.max)
                        else:
                            nc.vector.tensor_scalar_max(out=rl, in0=ps_m,
                                                        scalar1=0.0)
                        xp = xp4[:, dv2t, c, :]
                        nc.vector.tensor_tensor(out=xp, in0=rl,
                                                in1=Xn4[:, dv2t, c, :],
                                                op=OP.add)
                        sq = tmpp.tile([128, 512], BF16, tag="sq")
                        nc.vector.tensor_tensor(out=sq, in0=xp, in1=xp,
                                                op=OP.mult)
                        nc.tensor.matmul(st_x, lhsT=ones_b, rhs=xp,
                                         start=(dv2t == 0), stop=(dv2t == 3))
                        nc.tensor.matmul(st_s, lhsT=ones_b, rhs=sq,
                                         start=(dv2t == 0), stop=(dv2t == 3))
                    mu = sml.tile([1, 512], F32, tag="mu")
                    nc.vector.tensor_scalar_mul(out=mu, in0=st_x,
                                                scalar1=1.0 / D)
                    mu2 = sml.tile([1, 512], F32, tag="mu2")
                    nc.vector.tensor_tensor(out=mu2, in0=mu, in1=mu,
                                            op=OP.mult)
                    var = sml.tile([1, 512], F32, tag="var")
                    nc.vector.scalar_tensor_tensor(out=var, in0=st_s,
                                                   scalar=1.0 / D, in1=mu2,
                                                   op0=OP.mult,
                                                   op1=OP.subtract)
                    lnv = sml.tile([1, 512], F32, tag="lnv")
                    nc.scalar.activation(out=lnv, in_=var, func=AF.Ln,
                                         bias=eps1, scale=1.0)
                    rstd = sml.tile([1, 512], F32, tag="rstd")
                    nc.scalar.activation(out=rstd, in_=lnv, func=AF.Exp,
                                         scale=-0.5)
                    nmr = sml.tile([1, 512], F32, tag="nmr")
                    nc.vector.scalar_tensor_tensor(out=nmr, in0=mu,
                                                   scalar=-1.0, in1=rstd,
                                                   op0=OP.mult, op1=OP.mult)
                    ps_b = psS.tile([128, 1024], F32, tag="s")
                    nc.tensor.matmul(ps_b[:, 0:512], lhsT=ones_r, rhs=rstd, start=True, stop=True)
                    nc.tensor.matmul(ps_b[:, 512:1024], lhsT=ones_r, rhs=nmr, start=True, stop=True)
                    for dv2t in range(4):
                        t2 = tmpp.tile([128, 512], BF16, tag="t2")
                        nc.vector.tensor_tensor(out=t2, in0=xp4[:, dv2t, c, :],
                                                in1=ps_b[:, 0:512], op=OP.mult)
                        oo = o4[:, dv2t, c, :]
                        nc.vector.tensor_tensor(out=oo, in0=t2,
                                                in1=ps_b[:, 512:1024],
                                                op=OP.add)
                        if ln1_aff:
                            nc.vector.tensor_scalar(
                                out=oo, in0=oo,
                                scalar1=g14[:, dv2t:dv2t + 1],
                                scalar2=b14[:, dv2t:dv2t + 1],
                                op0=OP.mult, op1=OP.add)

                nc.sync.dma_start(out=dOut, in_=outT)

            if repeat == 1:
                body()
            else:
                with tc.For_i(0, repeat, 1):
                    body()

    nc.compile()
    return nc


def _host_T(x):
    # [n, d] f32 -> [128, 4, n] bf16 with d = dt*128 + p
    n = x.shape[0]
    return np.ascontiguousarray(
        x.T.reshape(4, 128, n).transpose(1, 0, 2).astype(NBF))


def _consts(Wq, Wk, Wv, Wo, flags, bq, bk, bv, bo, g0, b0, g1, b1):
    (bq_nz, bk_nz, bv_nz, bo_nz, ln0_aff, ln1_aff) = flags

    def wblk(W):
        # [512, 512] -> [128, 4kt, 4dvt, 128] bf16,
        # lhsT block [p, kt, dvt, c] = W[kt*128+p, dvt*128+c]
        return np.ascontiguousarray(
            np.asarray(W).reshape(4, 128, 4, 128).transpose(1, 0, 2, 3)
            .astype(NBF))

    c = {
        "WQ": wblk(Wq),
        "WK": wblk(Wk),
        "WO": wblk(Wo),
        "WV": np.ascontiguousarray(
            np.asarray(Wv).reshape(4, 128, 512).transpose(1, 0, 2).astype(NBF)),
    }
    def p4(v):
        return np.ascontiguousarray(
            np.asarray(v, np.float32).reshape(4, 128).T)
    if bq_nz: c["BQ4"] = p4(bq)
    if bk_nz: c["BK4"] = p4(bk)
    if bv_nz: c["BVB"] = np.ascontiguousarray(
        np.broadcast_to(np.asarray(bv, np.float32), (128, 512)))
    if bo_nz: c["BO4"] = p4(bo)
    if ln0_aff:
        c["G04"] = p4(g0)
        c["B04"] = p4(b0)
    if ln1_aff:
        c["G14"] = p4(g1)
        c["B14"] = p4(b1)
    return c


def make_in_maps(Q, K, Wq, bq, Wk, bk, Wv, bv, Wo, bo, g0, b0, g1, b1, flags):
    consts = _consts(Wq, Wk, Wv, Wo, flags, bq, bk, bv, bo, g0, b0, g1, b1)
    Qf = np.asarray(Q, np.float32).reshape(B, NQ, 512)
    Kf = np.asarray(K, np.float32).reshape(B, NK, 512)
    in_maps = []
    for ci in range(N_CORES):
        m = dict(consts)
        m["QT"] = _host_T(Qf[ci * BL:(ci + 1) * BL].reshape(N, 512))
        m["KT"] = _host_T(Kf[ci * BL:(ci + 1) * BL].reshape(N, 512))
        in_maps.append(m)
    return in_maps


def get_flags(bq, bk, bv, bo, g0, b0, g1, b1):
    return (bool(np.any(np.asarray(bq))), bool(np.any(np.asarray(bk))),
            bool(np.any(np.asarray(bv))), bool(np.any(np.asarray(bo))),
            bool(np.any(np.asarray(g0) != 1) or np.any(np.asarray(b0))),
            bool(np.any(np.asarray(g1) != 1) or np.any(np.asarray(b1))))


def get_program(flags, repeat=1):
    key = (flags, repeat)
    if key not in _cache:
        _cache[key] = _build(flags, repeat)
    return _cache[key]


def kernel(Q, K, Wq, bq, Wk, bk, Wv, bv, Wo, bo, g0, b0, g1, b1):
    flags = get_flags(bq, bk, bv, bo, g0, b0, g1, b1)
    nc = get_program(flags, repeat=1)
    in_maps = make_in_maps(Q, K, Wq, bq, Wk, bk, Wv, bv, Wo, bo,
                           g0, b0, g1, b1, flags)
    res = run_bass_kernel_spmd(nc, in_maps, list(range(N_CORES)))
    out = np.empty((B, NQ, D), np.float32)
    for ci in range(N_CORES):
        o = np.asarray(res.results[ci]["OUT"]).astype(np.float32)
        # [128, 4, N] -> [N, 512] with d = dt*128 + p
        out[ci * BL:(ci + 1) * BL] = (
            o.transpose(1, 0, 2).reshape(512, N).T.reshape(BL, NQ, D))
    return out
